# revision 26
# baseline (speedup 1.0000x reference)
"""VQ-VAE EMA codebook update kernel for 8 Trainium2 NeuronCores.

Strategy (data-parallel, per sharding hint):
- Shard x_flat rows 8 ways; replicate the codebook.
- Per core: G = x @ emb.T via float32r matmuls (PSUM fp32); DVE max/max_index
  extracts per-row top-8 values/indices (argmin of squared distance == argmax
  of G up to the e_sq/x_sq terms, which only matter for near-ties); ACT Sign
  builds an (inverted) one-hot used by 8 f32r matmuls to accumulate
  dw = onehot.T @ x in a persistent PSUM group; gpsimd gathers the quantized
  rows and computes the straight-through output x + (q - x) in fp32.
- Host: exact fp32 tie-break correction for rows whose top-2 G gap is inside
  a small window (the reference's fp32 rounding of x_sq - 2G + e_sq reorders
  near-ties; those rows' full distance rows are recomputed in numpy exactly
  as the CPU-jax reference computes them, which was measured bit-compatible),
  then bincount/EMA/losses/perplexity epilogue in reference-matching fp32.
"""
import os

import numpy as np

DECAY = 0.999
EPS = 1e-05
COMMITMENT_COST = 0.25
M, D = 1024, 256
N = 32 * 32 * 32          # 32768 rows
NCORES = 8
NSH = N // NCORES         # 4096 rows per core
NTILES = NSH // 128       # 32 tiles of 128 rows
W_G = 2e-4                # host-correction window on G gap

_compiled = {}


def _build():
    import concourse.bass as bass
    from concourse import bacc
    import concourse.mybir as mybir
    from concourse.tile import TileContext

    f32 = mybir.dt.float32
    f32r = mybir.dt.float32r
    u32 = mybir.dt.uint32
    AF = mybir.ActivationFunctionType

    nc = bacc.Bacc("TRN2", target_bir_lowering=False)
    d_x = nc.dram_tensor("d_x", [NSH, D], f32r, kind="ExternalInput")
    d_xT = nc.dram_tensor("d_xT", [D, NSH], f32r, kind="ExternalInput")
    d_eT = nc.dram_tensor("d_eT", [D, M], f32r, kind="ExternalInput")
    d_emb = nc.dram_tensor("d_emb", [M, D], f32, kind="ExternalInput")
    o_out1 = nc.dram_tensor("o_out1", [NSH, D], f32, kind="ExternalOutput")
    o_s8 = nc.dram_tensor("o_s8", [NSH, 8], f32, kind="ExternalOutput")
    o_i8 = nc.dram_tensor("o_i8", [NSH, 8], u32, kind="ExternalOutput")
    o_dwn = nc.dram_tensor("o_dwn", [M, D], f32, kind="ExternalOutput")

    xT_v = d_xT[:].rearrange("(c p) n -> p c n", p=128)   # [128, 2, NSH]
    eT_v = d_eT[:].rearrange("(c p) m -> p c m", p=128)   # [128, 2, M]

    with TileContext(nc) as tc:
        with tc.tile_pool(name="cst", bufs=1) as cst, \
             tc.tile_pool(name="io", bufs=4) as io, \
             tc.tile_pool(name="oh", bufs=6) as ohp, \
             tc.tile_pool(name="psG", bufs=2, space="PSUM") as psG, \
             tc.tile_pool(name="psW", bufs=1, space="PSUM") as psW:

            t_eT = cst.tile([128, 2, M], f32r)
            nc.sync.dma_start(t_eT[:, 0, :], eT_v[:, 0, :])
            nc.sync.dma_start(t_eT[:, 1, :], eT_v[:, 1, :])

            p_dw = psW.tile([128, 8, D], f32)   # 4 banks, persistent accumulator

            # accumulate per-tile top-8 stats in SBUF; single store at the end
            t_s8a = cst.tile([128, NTILES, 8], f32)
            t_i8a = cst.tile([128, NTILES, 8], u32)

            GRP = 4   # n-tiles per wide xT load (2KB DMA bursts)
            x_v = d_x[:].rearrange("(t p) d -> p t d", p=128)   # [128, NTILES, D]

            xTw = {}      # wide xT tiles by group
            xw = {}       # paired x tiles
            pG = {}       # per-tile G psum

            def emit_loads(t):
                if t % GRP == 0:
                    w = io.tile([128, 2, GRP * 128], f32r, tag="xT")
                    if t == 0:
                        # split so G(0) only waits on tile 0's slice
                        nc.sync.dma_start(w[:, :, 0:128], xT_v[:, :, 0:128])
                        nc.sync.dma_start(w[:, :, 128:GRP * 128],
                                          xT_v[:, :, 128:GRP * 128])
                    else:
                        nc.sync.dma_start(w[:], xT_v[:, :, t * 128:(t + GRP) * 128])
                    xTw[t // GRP] = w
                if t % 2 == 0:
                    w = io.tile([128, 2, D], f32r, tag="x")
                    # first pair via the idle Pool SWDGE queue (head latency)
                    eng = nc.gpsimd if t == 0 else nc.sync
                    eng.dma_start(w[:], x_v[:, t:t + 2, :])
                    xw[t // 2] = w

            def emit_G(t):
                g0 = (t % GRP) * 128
                p = psG.tile([128, M], f32, tag="G")   # 2 banks
                for c in range(2):      # stationary operand outer: 2 LDW/tile
                    for h in range(2):
                        nc.tensor.matmul(p[:, h * 512:(h + 1) * 512],
                                         xTw[t // GRP][:, c, g0:g0 + 128],
                                         t_eT[:, c, h * 512:(h + 1) * 512],
                                         start=(c == 0), stop=(c == 1))
                pG[t] = p

            emit_loads(0)
            emit_G(0)
            for t in range(NTILES):
                n0 = t * 128
                if t + 1 < NTILES:
                    emit_loads(t + 1)
                t_x = xw[t // 2][:, t % 2, :]
                p_G = pG.pop(t)

                # stage G into SBUF: the copy is p_G's ONLY reader, so the
                # PSUM slot frees immediately; DVE/ACT then read SBUF
                # (lower access overhead, 2x ACT mode for the Sign)
                t_Gs = ohp.tile([128, M], f32, tag="Gs")
                nc.scalar.copy(t_Gs[:], p_G[:])

                # prefetch next tile's G matmuls ahead of this tile's dw
                # matmuls in PE priority, so the p_G slot turns around fast
                if t + 1 < NTILES:
                    emit_G(t + 1)

                t_s8 = t_s8a[:, t, :]
                nc.vector.max(out=t_s8, in_=t_Gs[:])

                # ohdev = Sign(max0 - G): 0 at argmax, +1 elsewhere.
                # Issued before max_index so ACT only waits on the max, not
                # the whole DVE chain.
                t_oh = ohp.tile([128, M], f32r, tag="oh")
                nc.scalar.activation(t_oh[:], t_Gs[:], AF.Sign,
                                     bias=t_s8[:, 0:1], scale=-1.0)

                t_i8 = t_i8a[:, t, :]
                nc.vector.max_index(t_i8, t_s8, t_Gs[:])

                # dw accumulation: p_dw[c] += ohdev[:, c].T @ x  (= colsum - dw)
                for c in range(8):
                    # one accumulation group per PSUM bank: chunks 2b/2b+1
                    # share a bank, whose zero-region is cleared once by the
                    # even chunk's start; the odd chunk first-writes onto
                    # cleared has_written bits (overwrite semantics).
                    nc.tensor.matmul(p_dw[:, c, :],
                                     t_oh[:, c * 128:(c + 1) * 128],
                                     t_x,
                                     start=(t == 0 and c % 2 == 0),
                                     stop=(t == NTILES - 1 and c % 2 == 1))

                # gather quantized rows; straight-through output on gpsimd
                t_q = io.tile([128, D], f32, tag="q")
                nc.gpsimd.indirect_dma_start(
                    out=t_q[:], out_offset=None, in_=d_emb[:],
                    in_offset=bass.IndirectOffsetOnAxis(ap=t_i8[:, 0:1], axis=0))
                t_d = io.tile([128, D], f32, tag="d")
                nc.gpsimd.tensor_sub(t_d[:], t_q[:], t_x.bitcast(f32))
                if t % 2 == 0:
                    t_o1w = io.tile([128, 2, D], f32, tag="o1")
                nc.gpsimd.tensor_add(t_o1w[:, t % 2, :], t_x.bitcast(f32), t_d[:])

                if t % 2 == 1:
                    # paired store on the ACT HWDGE queue (SP queue is busier)
                    nc.scalar.dma_start(
                        o_out1[:].rearrange("(t p) d -> p t d", p=128)[:, t - 1:t + 1, :],
                        t_o1w[:])

            # batched stat stores (row n = t*128 + p)
            nc.sync.dma_start(o_s8[:].rearrange("(t p) k -> p t k", p=128), t_s8a[:])
            nc.sync.dma_start(o_i8[:].rearrange("(t p) k -> p t k", p=128), t_i8a[:])

            # flush dw accumulator
            t_dwo = ohp.tile([128, 8, D], f32, tag="dwo")
            nc.scalar.copy(t_dwo[:], p_dw[:])
            nc.sync.dma_start(o_dwn[:].rearrange("(c p) d -> p c d", p=128), t_dwo[:])

    nc.compile()
    return nc


def _get_nc():
    if "nc" not in _compiled:
        _compiled["nc"] = _build()
    return _compiled["nc"]


def _run_sim(nc, in_maps):
    from concourse.bass_interp import CoreSim
    outs = []
    for m in in_maps:
        sim = CoreSim(nc, trace=False)
        for k, v in m.items():
            sim.tensor(k)[:] = v
        sim.simulate()
        outs.append({k: np.array(sim.tensor(k))
                     for k in ("o_out1", "o_s8", "o_i8", "o_dwn")})
    return outs


def kernel(x, embedding, ema_count, ema_weight, _trace=False):
    from concourse import bass_utils

    x = np.ascontiguousarray(np.asarray(x, dtype=np.float32))
    emb = np.ascontiguousarray(np.asarray(embedding, dtype=np.float32))
    ema_count = np.asarray(ema_count, dtype=np.float32)
    ema_weight = np.asarray(ema_weight, dtype=np.float32)

    xf = x.reshape(N, D)
    eT = np.ascontiguousarray(emb.T)

    in_maps = []
    for k in range(NCORES):
        xs = xf[k * NSH:(k + 1) * NSH]
        in_maps.append({
            "d_x": xs,
            "d_xT": np.ascontiguousarray(xs.T),
            "d_eT": eT,
            "d_emb": emb,
        })

    nc = _get_nc()
    if os.environ.get("VQ_SIM"):
        outs = _run_sim(nc, in_maps)
        res = None
    else:
        res = bass_utils.run_bass_kernel_spmd(nc, in_maps,
                                              core_ids=list(range(NCORES)),
                                              trace=_trace)
        outs = res.results

    out1 = np.concatenate([o["o_out1"] for o in outs], axis=0)      # [N, D]
    s8 = np.concatenate([o["o_s8"] for o in outs], axis=0)          # [N, 8]
    i8 = np.concatenate([o["o_i8"] for o in outs], axis=0)          # [N, 8]
    # dw per core: dwn = colsum_core - dw_core  ->  dw = sum_k (colsum_k - dwn_k)
    dw = np.zeros((M, D), np.float64)
    for k, o in enumerate(outs):
        xs = xf[k * NSH:(k + 1) * NSH]
        colsum = xs.sum(0, dtype=np.float64)
        dw += colsum[None, :] - o["o_dwn"].astype(np.float64)

    idx = i8[:, 0].astype(np.int64)

    # --- host tie-break correction (replicates the reference's fp32 rounding) ---
    gap = s8[:, 0] - s8[:, 1]
    flagged = np.where(gap < W_G)[0]
    if flagged.size:
        xs_f = xf[flagged]
        x_sq_f = np.sum(xf * xf, axis=-1, dtype=np.float32)[flagged]
        e_sq = np.sum(emb * emb, axis=-1, dtype=np.float32)
        G_f = (xs_f @ emb.T).astype(np.float32)
        d_f = ((x_sq_f[:, None] - (np.float32(2.0) * G_f)) + e_sq[None, :]).astype(np.float32)
        idx_f = d_f.argmin(-1)
    else:
        idx_f = np.zeros(0, np.int64)

    # device-side exact ties in G added x_n into multiple codebook rows; remove extras
    tie_rows = np.where(s8[:, 0] == s8[:, 1])[0]
    for r in tie_rows:
        for k in range(1, 8):
            if s8[r, k] == s8[r, 0] and i8[r, k] != i8[r, 0]:
                dw[i8[r, k]] -= xf[r]
            if s8[r, k] != s8[r, 0]:
                break

    final_idx = idx.copy()
    final_idx[flagged] = idx_f
    changed = np.where(final_idx != idx)[0]
    for r in changed:
        dw[idx[r]] -= xf[r]
        dw[final_idx[r]] += xf[r]
        q_new = emb[final_idx[r]]
        out1[r] = (xf[r] + (q_new - xf[r]).astype(np.float32)).astype(np.float32)

    dw = dw.astype(np.float32)

    # --- epilogue (replicates reference fp32 ops elementwise) ---
    counts = np.bincount(final_idx, minlength=M).astype(np.float32)

    one_m_decay = np.float32(1.0 - DECAY)
    decay = np.float32(DECAY)
    new_count = decay * ema_count + one_m_decay * counts
    n_tot = np.float32(new_count.sum(dtype=np.float32))
    new_count = (new_count + np.float32(EPS)) / np.float32(n_tot + M * EPS) * n_tot
    new_weight = decay * ema_weight + one_m_decay * dw
    new_embedding = new_weight / new_count[:, None]

    sq_sum = 0.0
    for k in range(0, N, 4096):
        qk = emb[final_idx[k:k + 4096]]
        dk = xf[k:k + 4096].astype(np.float64) - qk.astype(np.float64)
        sq_sum += float(np.einsum("nd,nd->", dk, dk))
    e_latent = np.float32(sq_sum / (N * D))
    commitment_loss = np.float32(COMMITMENT_COST) * e_latent
    codebook_loss = e_latent

    avg_probs = counts / np.float32(N)
    perplexity = np.float32(
        np.exp(-np.sum(avg_probs * np.log(avg_probs + np.float32(1e-10)),
                       dtype=np.float32)))

    quantized_st = out1.reshape(x.shape)
    if _trace and res is not None:
        kernel._last_exec_ns = res.exec_time_ns
    return (quantized_st, commitment_loss, codebook_loss, perplexity,
            new_embedding, new_count, new_weight)
